# revision 19
# baseline (speedup 1.0000x reference)
"""MoE FFN (8 experts, top-2) Trainium2 kernel — expert-parallel over 8 NeuronCores.

Strategy:
  - Host: router (softmax/top-2/aux-loss, ~0.05% of FLOPs), token dispatch by
    expert id (the sharding itself), final weighted scatter-add combine.
  - Device (one NEFF, SPMD on 8 cores, per-core inputs): core e holds expert
    e's weights (bf16) and computes silu(x@Wg.T)*(x@Wu.T)@Wd.T for the tokens
    routed to expert e, scaled by the per-token combine weight.
  - Layout: contraction always on the partition axis. Phase 1 computes
    G'=[I,C] (intermediate on partitions) so phase 2 can consume it as lhsT
    directly — no transposes anywhere.
"""

import math
import os
import sys

import numpy as np

for _p in ("/opt/trn_rl_repo", "/root/.axon_site/_ro/trn_rl_repo"):
    if os.path.isdir(_p) and _p not in sys.path:
        sys.path.append(_p)

import ml_dtypes  # noqa: E402
import concourse.bass as bass  # noqa: E402
import concourse.mybir as mybir  # noqa: E402
import concourse.tile as tile  # noqa: E402
from concourse.mybir import SyncInfo  # noqa: E402
from bass_rust import ScopedClock  # noqa: E402

H = 1024
I = 2816
E = 8
TOP_K = 2
NCORES = 8
P = 128
KO = H // P          # 8 k-subtiles over H
MI = I // P          # 22 tiles over I
NH = H // 512        # 2 output column chunks
BF16 = mybir.dt.bfloat16
F32 = mybir.dt.float32

_BF16_NP = ml_dtypes.bfloat16


# ---------------------------------------------------------------------------
# Workarounds for this container's walrus build (max ONE sync wait per inst).
# ---------------------------------------------------------------------------

def _patched_drain_and_barrier(self, tick_clock, wait_clock):
    nc = self.nc
    drain_inst = nc.sync.drain()
    wait_clock.add_sem_waits(
        drain_inst.ins, ScopedClock({None: tick_clock.global_clock})
    )
    nc.all_engine_barrier()
    popped = nc._tile_sem_poison_stack.pop()
    assert popped is self._sem_poison
    nc.clear_and_free_semaphores(list(self.sems.allocated().values()))
    nc.all_engine_barrier()


tile.TileContext._drain_and_barrier = _patched_drain_and_barrier


def _split_multiwaits(nc):
    """Walrus here rejects >1 sync wait per instruction; split extras onto
    preceding same-engine NoOps (engine streams are in-order, so equivalent)."""
    nsplit = 0
    for f in nc.m.functions:
        for bb in f.blocks:
            insts = list(bb.instructions)
            out = []
            changed = False
            for inst in insts:
                si = inst.sync_info
                if si is not None and len(si.on_wait) > 1:
                    waits = list(si.on_wait)
                    for w in waits[:-1]:
                        nop = mybir.InstNoOp(
                            name=f"wsplit_{nsplit}",
                            opcode="NoOp",
                            engine=inst.engine,
                            sync_info=SyncInfo(on_wait=[w], on_update=[]),
                        )
                        out.append(nop)
                        nsplit += 1
                    inst.sync_info = SyncInfo(
                        on_wait=[waits[-1]], on_update=list(si.on_update)
                    )
                    changed = True
                out.append(inst)
            if changed:
                bb.instructions = out
    return nsplit


# ---------------------------------------------------------------------------
# Device kernel
# ---------------------------------------------------------------------------

def _chunks_of(C, nchunk=512):
    """Token-column chunks (matmul moving dim): nchunk-size then a 128-mult tail."""
    out = []
    c0 = 0
    while c0 < C:
        n = min(nchunk, C - c0)
        out.append((c0, n))
        c0 += n
    return out


def _build_bass(
    C,
    *,
    nchunk=512,
    abufs=1,
    xbufs=2,
    pg_bufs=2,
    pu_bufs=2,
    py_bufs=3,
    sbufs=4,
    ybufs=4,
):
    """Expert FFN for C tokens (C a multiple of 128). Inputs per core:
    xt [H,C] bf16, wgt/wut [H,I] bf16, wdt [I,H] bf16, cw [128,C/128] f32.
    Output y [C,H] f32 (rows pre-scaled by combine weight)."""
    assert C % P == 0
    nc = bass.Bass(
        "TRN2", target_bir_lowering=False, debug=False, num_devices=NCORES
    )
    xt = nc.dram_tensor("xt", [H, C], BF16, kind="ExternalInput")
    wgt = nc.dram_tensor("wgt", [H, I], BF16, kind="ExternalInput")
    wut = nc.dram_tensor("wut", [H, I], BF16, kind="ExternalInput")
    wdt = nc.dram_tensor("wdt", [I, H], BF16, kind="ExternalInput")
    cw = nc.dram_tensor("cw", [P, C // P], F32, kind="ExternalInput")
    y = nc.dram_tensor("y", [C, H], F32, kind="ExternalOutput")

    xt_r = xt.rearrange("(ko p) c -> p ko c", p=P)
    wgt_r = wgt.rearrange("(ko p) i -> p ko i", p=P)
    wut_r = wut.rearrange("(ko p) i -> p ko i", p=P)
    wdt_r = wdt.rearrange("(mi p) h -> mi p h", p=P)
    y_r = y.rearrange("(j p) h -> j p h", p=P)

    with tile.TileContext(nc) as tc:
        with (
            tc.tile_pool(name="wpool", bufs=1) as wpool,
            tc.tile_pool(name="xpool", bufs=xbufs) as xpool,
            tc.tile_pool(name="apool", bufs=abufs) as apool,
            tc.tile_pool(name="spool", bufs=sbufs) as spool,
            tc.tile_pool(name="ypool", bufs=ybufs) as ypool,
            tc.tile_pool(name="cpool", bufs=1) as cpool,
            tc.tile_pool(name="psg", bufs=pg_bufs, space="PSUM") as psg,
            tc.tile_pool(name="psu", bufs=pu_bufs, space="PSUM") as psu,
            tc.tile_pool(name="psy", bufs=py_bufs, space="PSUM") as psy,
        ):
            # Per-mb weight tiles (all k in one tile, one DMA each) so the
            # first m-iterations' weights land quickly: kills the ~46us PE
            # startup bubble waiting on the full 17MB weight load.
            MB = 2
            NMB = MI // MB
            chunks = _chunks_of(C, nchunk)

            def _load_wblock(mb):
                sl = slice(mb * MB * P, (mb + 1) * MB * P)
                wgb = wpool.tile(
                    [P, KO, MB * P], BF16, name=f"wg{mb}", tag=f"wg{mb}"
                )
                nc.sync.dma_start(wgb, wgt_r[:, :, sl])
                wub = wpool.tile(
                    [P, KO, MB * P], BF16, name=f"wu{mb}", tag=f"wu{mb}"
                )
                nc.sync.dma_start(wub, wut_r[:, :, sl])
                return wgb, wub

            def _load_xtile(c0, nch):
                xtile = xpool.tile([P, KO, nchunk], BF16, tag="xtile")
                nc.sync.dma_start(
                    xtile[:, :, :nch], xt_r[:, :, c0 : c0 + nch]
                )
                return xtile

            # DMA issue order = consumption order: first weight block, first
            # x chunk, then the remaining weights (rest of x streams in-loop).
            wg = [None] * NMB
            wu = [None] * NMB
            wg[0], wu[0] = _load_wblock(0)
            xtile0 = _load_xtile(*chunks[0])
            for mb in range(1, NMB):
                wg[mb], wu[mb] = _load_wblock(mb)
            cwt = cpool.tile([P, C // P], F32, name="cwt")
            nc.sync.dma_start(cwt, cw[:, :])
            wd = []
            for m in range(MI):
                wdm = wpool.tile([P, H], BF16, name=f"wd{m}", tag=f"wd{m}")
                nc.sync.dma_start(wdm, wdt_r[m])
                wd.append(wdm)

            for ci, (c0, nch) in enumerate(chunks):
                xtile = xtile0 if ci == 0 else _load_xtile(c0, nch)
                a = apool.tile([P, MI, nchunk], BF16, tag="a")
                for m in range(MI):
                    mb, mo = divmod(m, MB)
                    msl = slice(mo * P, (mo + 1) * P)
                    pg = psg.tile([P, nchunk], F32, tag="pg")
                    pu = psu.tile([P, nchunk], F32, tag="pu")
                    for k in range(KO):
                        nc.tensor.matmul(
                            pg[:, :nch],
                            wg[mb][:, k, msl],
                            xtile[:, k, :nch],
                            start=(k == 0),
                            stop=(k == KO - 1),
                        )
                    for k in range(KO):
                        nc.tensor.matmul(
                            pu[:, :nch],
                            wu[mb][:, k, msl],
                            xtile[:, k, :nch],
                            start=(k == 0),
                            stop=(k == KO - 1),
                        )
                    s = spool.tile([P, nchunk], BF16, tag="s")
                    nc.scalar.activation(
                        s[:, :nch],
                        pg[:, :nch],
                        mybir.ActivationFunctionType.Silu,
                    )
                    nc.vector.tensor_mul(
                        a[:, m, :nch], s[:, :nch], pu[:, :nch]
                    )
                for mc in range(nch // P):
                    jglob = c0 // P + mc
                    for nh in range(NH):
                        py = psy.tile([P, 512], F32, tag="py")
                        for m in range(MI):
                            nc.tensor.matmul(
                                py,
                                a[:, m, mc * P : (mc + 1) * P],
                                wd[m][:, nh * 512 : (nh + 1) * 512],
                                start=(m == 0),
                                stop=(m == MI - 1),
                            )
                        yo = ypool.tile([P, 512], F32, tag="yo")
                        nc.vector.tensor_scalar_mul(
                            yo, py, cwt[:, jglob : jglob + 1]
                        )
                        nc.sync.dma_start(
                            y_r[jglob, :, nh * 512 : (nh + 1) * 512], yo
                        )

    _split_multiwaits(nc)
    return nc


_NC_CACHE = {}


def _get_bass(C):
    if C not in _NC_CACHE:
        _NC_CACHE[C] = _build_bass(C)
    return _NC_CACHE[C]


# ---------------------------------------------------------------------------
# Host orchestration
# ---------------------------------------------------------------------------

def _route(x, gate_w):
    """fp32 router identical to the reference (softmax -> top-2 -> renorm)."""
    T = x.shape[0]
    logits = (x @ gate_w.T).astype(np.float32)
    m = logits.max(axis=-1, keepdims=True)
    p = np.exp(logits - m)
    p = p / p.sum(axis=-1, keepdims=True)
    idx = np.argsort(-p, axis=-1, kind="stable")[:, :TOP_K]
    w = np.take_along_axis(p, idx, axis=-1)
    w = (w / np.clip(w.sum(-1, keepdims=True), 1e-8, None)).astype(np.float32)

    top1 = idx[:, 0]
    counts1 = np.bincount(top1, minlength=E).astype(np.float32)
    fraction_routed = counts1 / np.float32(T)
    mean_router_prob = p.mean(axis=0)
    aux_loss = np.float32(E * np.sum(fraction_routed * mean_router_prob))
    return idx, w, aux_loss


def _dispatch(idx, w):
    """Group (token, k) pairs by expert. Device capacity C is capped at the
    mean load (rounded up to 128); the few overflow pairs of hot experts
    (<1% at balanced routing) are returned for exact fp32 host compute, so
    every core runs a zero-padding-free uniform workload."""
    T = idx.shape[0]
    flat_e = idx.reshape(-1)
    flat_t = np.repeat(np.arange(T, dtype=np.int64), TOP_K)
    flat_w = w.reshape(-1)
    order = np.argsort(flat_e, kind="stable")
    se, st, sw = flat_e[order], flat_t[order], flat_w[order]
    counts = np.bincount(flat_e, minlength=E)
    cmax = ((max(int(counts.max()), P) + P - 1) // P) * P
    cmean = ((int(math.ceil(counts.mean())) + P - 1) // P) * P
    C = min(cmax, cmean)
    # guard: if routing is very imbalanced, don't shift real work to host
    if (counts - C).clip(min=0).sum() > 0.02 * T * TOP_K:
        C = cmax
    tok = np.zeros((E, C), np.int64)
    cwt = np.zeros((E, C), np.float32)
    starts = np.concatenate([[0], np.cumsum(counts)])
    ov_t, ov_e, ov_w = [], [], []
    for e in range(E):
        n = int(counts[e])
        ndev = min(n, C)
        tok[e, :ndev] = st[starts[e] : starts[e] + ndev]
        cwt[e, :ndev] = sw[starts[e] : starts[e] + ndev]
        if n > C:
            ov_t.append(st[starts[e] + C : starts[e] + n])
            ov_e.append(np.full(n - C, e, np.int64))
            ov_w.append(sw[starts[e] + C : starts[e] + n])
    if ov_t:
        overflow = (
            np.concatenate(ov_t),
            np.concatenate(ov_e),
            np.concatenate(ov_w),
        )
    else:
        overflow = None
    return tok, cwt, np.minimum(counts, C), C, overflow


_LAST_RUN = {}  # exposed for test.py timing


def kernel(hidden_states, gate_w, w_gate, w_up, w_down):
    from concourse.bass_utils import run_bass_kernel_spmd

    hs = np.asarray(hidden_states, dtype=np.float32)
    b, s, h = hs.shape
    assert h == H
    x = hs.reshape(-1, H)
    T = x.shape[0]

    gate_w = np.asarray(gate_w, dtype=np.float32)
    w_gate = np.asarray(w_gate, dtype=np.float32)
    w_up = np.asarray(w_up, dtype=np.float32)
    w_down = np.asarray(w_down, dtype=np.float32)

    idx, w, aux_loss = _route(x, gate_w)
    tok, cwt, counts, C, overflow = _dispatch(idx, w)

    nc = _get_bass(C)

    in_maps = []
    for e in range(E):
        xg = x[tok[e]]  # [C, H] f32
        in_maps.append(
            {
                "xt": np.ascontiguousarray(xg.T.astype(_BF16_NP)),
                "wgt": np.ascontiguousarray(w_gate[e].T.astype(_BF16_NP)),
                "wut": np.ascontiguousarray(w_up[e].T.astype(_BF16_NP)),
                "wdt": np.ascontiguousarray(w_down[e].T.astype(_BF16_NP)),
                "cw": np.ascontiguousarray(
                    cwt[e].reshape(C // P, P).T
                ),
                "y": None,  # placeholder, removed below
            }
        )
    for m in in_maps:
        del m["y"]

    res = run_bass_kernel_spmd(
        nc, in_maps, core_ids=list(range(NCORES)), trace=False
    )
    _LAST_RUN["nc"] = nc
    _LAST_RUN["in_maps"] = in_maps
    _LAST_RUN["C"] = C

    out = np.zeros((T, H), np.float32)
    for e in range(E):
        n = int(counts[e])
        ye = res.results[e]["y"]  # [C, H] f32, rows already scaled
        np.add.at(out, tok[e, :n], ye[:n])

    if overflow is not None:
        ov_t, ov_e, ov_w = overflow
        for e in np.unique(ov_e):
            sel = ov_e == e
            xt_ = x[ov_t[sel]]
            g = xt_ @ w_gate[e].T
            u = xt_ @ w_up[e].T
            yv = (g / (1.0 + np.exp(-g)) * u) @ w_down[e].T
            np.add.at(out, ov_t[sel], ov_w[sel][:, None] * yv)

    return out.reshape(b, s, h), aux_loss


# revision 21
# speedup vs baseline: 1.2951x; 1.2951x over previous
"""MoE FFN (8 experts, top-2) Trainium2 kernel — expert-parallel over 8 NeuronCores.

Strategy:
  - Host: router (softmax/top-2/aux-loss, ~0.05% of FLOPs), token dispatch by
    expert id (the sharding itself), final weighted scatter-add combine.
  - Device (one NEFF, SPMD on 8 cores, per-core inputs): core e holds expert
    e's weights (bf16) and computes silu(x@Wg.T)*(x@Wu.T)@Wd.T for the tokens
    routed to expert e, scaled by the per-token combine weight.
  - Layout: contraction always on the partition axis. Phase 1 computes
    G'=[I,C] (intermediate on partitions) so phase 2 can consume it as lhsT
    directly — no transposes anywhere.
"""

import math
import os
import sys

import numpy as np

for _p in ("/opt/trn_rl_repo", "/root/.axon_site/_ro/trn_rl_repo"):
    if os.path.isdir(_p) and _p not in sys.path:
        sys.path.append(_p)

import ml_dtypes  # noqa: E402
import concourse.bass as bass  # noqa: E402
import concourse.mybir as mybir  # noqa: E402
import concourse.tile as tile  # noqa: E402
from concourse.mybir import SyncInfo  # noqa: E402
from bass_rust import ScopedClock  # noqa: E402

H = 1024
I = 2816
E = 8
TOP_K = 2
NCORES = 8
P = 128
KO = H // P          # 8 k-subtiles over H
MI = I // P          # 22 tiles over I
NH = H // 512        # 2 output column chunks
BF16 = mybir.dt.bfloat16
F32 = mybir.dt.float32

_BF16_NP = ml_dtypes.bfloat16


# ---------------------------------------------------------------------------
# Workarounds for this container's walrus build (max ONE sync wait per inst).
# ---------------------------------------------------------------------------

def _patched_drain_and_barrier(self, tick_clock, wait_clock):
    nc = self.nc
    drain_inst = nc.sync.drain()
    wait_clock.add_sem_waits(
        drain_inst.ins, ScopedClock({None: tick_clock.global_clock})
    )
    nc.all_engine_barrier()
    popped = nc._tile_sem_poison_stack.pop()
    assert popped is self._sem_poison
    nc.clear_and_free_semaphores(list(self.sems.allocated().values()))
    nc.all_engine_barrier()


tile.TileContext._drain_and_barrier = _patched_drain_and_barrier


def _split_multiwaits(nc):
    """Walrus here rejects >1 sync wait per instruction; split extras onto
    preceding same-engine NoOps (engine streams are in-order, so equivalent)."""
    nsplit = 0
    for f in nc.m.functions:
        for bb in f.blocks:
            insts = list(bb.instructions)
            out = []
            changed = False
            for inst in insts:
                si = inst.sync_info
                if si is not None and len(si.on_wait) > 1:
                    waits = list(si.on_wait)
                    for w in waits[:-1]:
                        nop = mybir.InstNoOp(
                            name=f"wsplit_{nsplit}",
                            opcode="NoOp",
                            engine=inst.engine,
                            sync_info=SyncInfo(on_wait=[w], on_update=[]),
                        )
                        out.append(nop)
                        nsplit += 1
                    inst.sync_info = SyncInfo(
                        on_wait=[waits[-1]], on_update=list(si.on_update)
                    )
                    changed = True
                out.append(inst)
            if changed:
                bb.instructions = out
    return nsplit


# ---------------------------------------------------------------------------
# Device kernel
# ---------------------------------------------------------------------------

def _chunks_of(C, nchunk=512):
    """Token-column chunks (matmul moving dim): nchunk-size then a 128-mult tail."""
    out = []
    c0 = 0
    while c0 < C:
        n = min(nchunk, C - c0)
        out.append((c0, n))
        c0 += n
    return out


def _build_bass(
    C,
    *,
    nchunk=512,
    abufs=1,
    xbufs=2,
    pg_bufs=2,
    pu_bufs=2,
    py_bufs=3,
    sbufs=4,
    ybufs=4,
):
    """Expert FFN for C tokens (C a multiple of 128). Inputs per core:
    xt [H,C] bf16, wgt/wut [H,I] bf16, wdt [I,H] bf16, cw [128,C/128] f32.
    Output y [C,H] f32 (rows pre-scaled by combine weight)."""
    assert C % P == 0
    nc = bass.Bass(
        "TRN2", target_bir_lowering=False, debug=False, num_devices=NCORES
    )
    xt = nc.dram_tensor("xt", [H, C], BF16, kind="ExternalInput")
    wgt = nc.dram_tensor("wgt", [H, I], BF16, kind="ExternalInput")
    wut = nc.dram_tensor("wut", [H, I], BF16, kind="ExternalInput")
    wdt = nc.dram_tensor("wdt", [I, H], BF16, kind="ExternalInput")
    cw = nc.dram_tensor("cw", [P, C // P], F32, kind="ExternalInput")
    y = nc.dram_tensor("y", [C, H], F32, kind="ExternalOutput")

    xt_r = xt.rearrange("(ko p) c -> p ko c", p=P)
    wgt_r = wgt.rearrange("(ko p) i -> p ko i", p=P)
    wut_r = wut.rearrange("(ko p) i -> p ko i", p=P)
    wdt_r = wdt.rearrange("(mi p) h -> mi p h", p=P)
    y_r = y.rearrange("(j p) h -> j p h", p=P)

    with tile.TileContext(nc) as tc:
        with (
            tc.tile_pool(name="wpool", bufs=1) as wpool,
            tc.tile_pool(name="xpool", bufs=xbufs) as xpool,
            tc.tile_pool(name="apool", bufs=abufs) as apool,
            tc.tile_pool(name="spool", bufs=sbufs) as spool,
            tc.tile_pool(name="ypool", bufs=ybufs) as ypool,
            tc.tile_pool(name="cpool", bufs=1) as cpool,
            tc.tile_pool(name="psg", bufs=pg_bufs, space="PSUM") as psg,
            tc.tile_pool(name="psu", bufs=pu_bufs, space="PSUM") as psu,
            tc.tile_pool(name="psy", bufs=py_bufs, space="PSUM") as psy,
        ):
            # Per-mb weight tiles (all k in one tile, one DMA each) so the
            # first m-iterations' weights land quickly: kills the ~46us PE
            # startup bubble waiting on the full 17MB weight load.
            MB = 2
            NMB = MI // MB
            chunks = _chunks_of(C, nchunk)

            def _load_wblock(mb):
                sl = slice(mb * MB * P, (mb + 1) * MB * P)
                wgb = wpool.tile(
                    [P, KO, MB * P], BF16, name=f"wg{mb}", tag=f"wg{mb}"
                )
                nc.sync.dma_start(wgb, wgt_r[:, :, sl])
                wub = wpool.tile(
                    [P, KO, MB * P], BF16, name=f"wu{mb}", tag=f"wu{mb}"
                )
                nc.sync.dma_start(wub, wut_r[:, :, sl])
                return wgb, wub

            def _load_xtile(c0, nch):
                xtile = xpool.tile([P, KO, nchunk], BF16, tag="xtile")
                nc.sync.dma_start(
                    xtile[:, :, :nch], xt_r[:, :, c0 : c0 + nch]
                )
                return xtile

            # DMA issue order = consumption order. First weight block and
            # first x chunk are split per-k (plain outer-dim 2D row-slices of
            # the raw DRAM tensors — the AP class verified safe on HW) so the
            # very first matmuls start ~8us earlier.
            wg = [None] * NMB
            wu = [None] * NMB
            c00, nch0 = chunks[0]
            wg0k, x0k, wu0k = [], [], []
            for k in range(KO):
                hrow = slice(k * P, (k + 1) * P)
                wgp = wpool.tile(
                    [P, MB * P], BF16, name=f"wg0k{k}", tag=f"wg0k{k}"
                )
                nc.sync.dma_start(wgp, wgt[hrow, 0 : MB * P])
                wg0k.append(wgp)
                xp = xpool.tile([P, nchunk], BF16, tag=f"x0k{k}", bufs=1)
                nc.sync.dma_start(xp[:, :nch0], xt[hrow, c00 : c00 + nch0])
                x0k.append(xp)
            for k in range(KO):
                hrow = slice(k * P, (k + 1) * P)
                wup = wpool.tile(
                    [P, MB * P], BF16, name=f"wu0k{k}", tag=f"wu0k{k}"
                )
                nc.sync.dma_start(wup, wut[hrow, 0 : MB * P])
                wu0k.append(wup)
            for mb in range(1, NMB):
                wg[mb], wu[mb] = _load_wblock(mb)
            cwt = cpool.tile([P, C // P], F32, name="cwt")
            nc.sync.dma_start(cwt, cw[:, :])
            wd = []
            for m in range(MI):
                wdm = wpool.tile([P, H], BF16, name=f"wd{m}", tag=f"wd{m}")
                nc.sync.dma_start(wdm, wdt_r[m])
                wd.append(wdm)

            for ci, (c0, nch) in enumerate(chunks):
                xtile = None if ci == 0 else _load_xtile(c0, nch)
                a = apool.tile([P, MI, nchunk], BF16, tag="a")
                for m in range(MI):
                    mb, mo = divmod(m, MB)
                    msl = slice(mo * P, (mo + 1) * P)
                    pg = psg.tile([P, nchunk], F32, tag="pg")
                    pu = psu.tile([P, nchunk], F32, tag="pu")
                    for k in range(KO):
                        nc.tensor.matmul(
                            pg[:, :nch],
                            wg0k[k][:, msl] if mb == 0 else wg[mb][:, k, msl],
                            x0k[k][:, :nch] if ci == 0 else xtile[:, k, :nch],
                            start=(k == 0),
                            stop=(k == KO - 1),
                        )
                    for k in range(KO):
                        nc.tensor.matmul(
                            pu[:, :nch],
                            wu0k[k][:, msl] if mb == 0 else wu[mb][:, k, msl],
                            x0k[k][:, :nch] if ci == 0 else xtile[:, k, :nch],
                            start=(k == 0),
                            stop=(k == KO - 1),
                        )
                    s = spool.tile([P, nchunk], BF16, tag="s")
                    nc.scalar.activation(
                        s[:, :nch],
                        pg[:, :nch],
                        mybir.ActivationFunctionType.Silu,
                    )
                    nc.vector.tensor_mul(
                        a[:, m, :nch], s[:, :nch], pu[:, :nch]
                    )
                for mc in range(nch // P):
                    jglob = c0 // P + mc
                    for nh in range(NH):
                        py = psy.tile([P, 512], F32, tag="py")
                        for m in range(MI):
                            nc.tensor.matmul(
                                py,
                                a[:, m, mc * P : (mc + 1) * P],
                                wd[m][:, nh * 512 : (nh + 1) * 512],
                                start=(m == 0),
                                stop=(m == MI - 1),
                            )
                        yo = ypool.tile([P, 512], F32, tag="yo")
                        nc.vector.tensor_scalar_mul(
                            yo, py, cwt[:, jglob : jglob + 1]
                        )
                        nc.sync.dma_start(
                            y_r[jglob, :, nh * 512 : (nh + 1) * 512], yo
                        )

    _split_multiwaits(nc)
    return nc


_NC_CACHE = {}


def _get_bass(C):
    if C not in _NC_CACHE:
        _NC_CACHE[C] = _build_bass(C)
    return _NC_CACHE[C]


# ---------------------------------------------------------------------------
# Host orchestration
# ---------------------------------------------------------------------------

def _route(x, gate_w):
    """fp32 router identical to the reference (softmax -> top-2 -> renorm)."""
    T = x.shape[0]
    logits = (x @ gate_w.T).astype(np.float32)
    m = logits.max(axis=-1, keepdims=True)
    p = np.exp(logits - m)
    p = p / p.sum(axis=-1, keepdims=True)
    idx = np.argsort(-p, axis=-1, kind="stable")[:, :TOP_K]
    w = np.take_along_axis(p, idx, axis=-1)
    w = (w / np.clip(w.sum(-1, keepdims=True), 1e-8, None)).astype(np.float32)

    top1 = idx[:, 0]
    counts1 = np.bincount(top1, minlength=E).astype(np.float32)
    fraction_routed = counts1 / np.float32(T)
    mean_router_prob = p.mean(axis=0)
    aux_loss = np.float32(E * np.sum(fraction_routed * mean_router_prob))
    return idx, w, aux_loss


def _dispatch(idx, w):
    """Group (token, k) pairs by expert. Device capacity C is capped at the
    mean load (rounded up to 128); the few overflow pairs of hot experts
    (<1% at balanced routing) are returned for exact fp32 host compute, so
    every core runs a zero-padding-free uniform workload."""
    T = idx.shape[0]
    flat_e = idx.reshape(-1)
    flat_t = np.repeat(np.arange(T, dtype=np.int64), TOP_K)
    flat_w = w.reshape(-1)
    order = np.argsort(flat_e, kind="stable")
    se, st, sw = flat_e[order], flat_t[order], flat_w[order]
    counts = np.bincount(flat_e, minlength=E)
    cmax = ((max(int(counts.max()), P) + P - 1) // P) * P
    cmean = ((int(math.ceil(counts.mean())) + P - 1) // P) * P
    C = min(cmax, cmean)
    # guard: if routing is very imbalanced, don't shift real work to host
    if (counts - C).clip(min=0).sum() > 0.02 * T * TOP_K:
        C = cmax
    tok = np.zeros((E, C), np.int64)
    cwt = np.zeros((E, C), np.float32)
    starts = np.concatenate([[0], np.cumsum(counts)])
    ov_t, ov_e, ov_w = [], [], []
    for e in range(E):
        n = int(counts[e])
        ndev = min(n, C)
        tok[e, :ndev] = st[starts[e] : starts[e] + ndev]
        cwt[e, :ndev] = sw[starts[e] : starts[e] + ndev]
        if n > C:
            ov_t.append(st[starts[e] + C : starts[e] + n])
            ov_e.append(np.full(n - C, e, np.int64))
            ov_w.append(sw[starts[e] + C : starts[e] + n])
    if ov_t:
        overflow = (
            np.concatenate(ov_t),
            np.concatenate(ov_e),
            np.concatenate(ov_w),
        )
    else:
        overflow = None
    return tok, cwt, np.minimum(counts, C), C, overflow


_LAST_RUN = {}  # exposed for test.py timing


def kernel(hidden_states, gate_w, w_gate, w_up, w_down):
    from concourse.bass_utils import run_bass_kernel_spmd

    hs = np.asarray(hidden_states, dtype=np.float32)
    b, s, h = hs.shape
    assert h == H
    x = hs.reshape(-1, H)
    T = x.shape[0]

    gate_w = np.asarray(gate_w, dtype=np.float32)
    w_gate = np.asarray(w_gate, dtype=np.float32)
    w_up = np.asarray(w_up, dtype=np.float32)
    w_down = np.asarray(w_down, dtype=np.float32)

    idx, w, aux_loss = _route(x, gate_w)
    tok, cwt, counts, C, overflow = _dispatch(idx, w)

    nc = _get_bass(C)

    in_maps = []
    for e in range(E):
        xg = x[tok[e]]  # [C, H] f32
        in_maps.append(
            {
                "xt": np.ascontiguousarray(xg.T.astype(_BF16_NP)),
                "wgt": np.ascontiguousarray(w_gate[e].T.astype(_BF16_NP)),
                "wut": np.ascontiguousarray(w_up[e].T.astype(_BF16_NP)),
                "wdt": np.ascontiguousarray(w_down[e].T.astype(_BF16_NP)),
                "cw": np.ascontiguousarray(
                    cwt[e].reshape(C // P, P).T
                ),
                "y": None,  # placeholder, removed below
            }
        )
    for m in in_maps:
        del m["y"]

    res = run_bass_kernel_spmd(
        nc, in_maps, core_ids=list(range(NCORES)), trace=False
    )
    _LAST_RUN["nc"] = nc
    _LAST_RUN["in_maps"] = in_maps
    _LAST_RUN["C"] = C

    out = np.zeros((T, H), np.float32)
    for e in range(E):
        n = int(counts[e])
        ye = res.results[e]["y"]  # [C, H] f32, rows already scaled
        np.add.at(out, tok[e, :n], ye[:n])

    if overflow is not None:
        ov_t, ov_e, ov_w = overflow
        for e in np.unique(ov_e):
            sel = ov_e == e
            xt_ = x[ov_t[sel]]
            g = xt_ @ w_gate[e].T
            u = xt_ @ w_up[e].T
            yv = (g / (1.0 + np.exp(-g)) * u) @ w_down[e].T
            np.add.at(out, ov_t[sel], ov_w[sel][:, None] * yv)

    return out.reshape(b, s, h), aux_loss
